# revision 55
# baseline (speedup 1.0000x reference)
"""Trainium2 Bass kernel for nn_AllAmplitude (helicity-amplitude intensity).

Math: the reference contracts two spin-1 Wigner-D matrices per (resonance,
event) with a Breit-Wigner weight and sums |amp|^2 over external helicities.
Because D1 @ D2 = D^1(U1 U2) for the SU(2) elements U1, U2 of the two
rotations, the whole intensity collapses to

  I = 7 sum_r |w_r|^2
    + sum_{r<r'} 2 Re(w_r conj(w_r')) (10 Re(av)^2 + 2 Im(av)^2 - 3)

with av = conj(a_r') a_r + b_r' conj(b_r), (a, b) the Cayley-Klein parameters
of the composed rotation, and w_r the complex Breit-Wigner weight.

v2 changes vs the original kernel:
 - 4 fused range-reduction ops (fp16 out) instead of 8: the cosine variants
   come from cos(2*pi*f) = sin(pi/2 - 2*pi*|f|), with |f| produced by a
   single packed 4x-mode int16 AND-mask tensor_scalar; the C/D sign flips
   fold into the ACT sin scale (-2*pi).
 - |w_r|^2 = |coef_r|^2 / den_r: the diagonal term is an exact 2x-mode
   tensor_scalar on the reciprocal (replaces a 1x custom square op).
 - DMA issue split across Sync (alpha1, alpha2, beta1, beta2, m) and the
   Scalar engine (gamma1, gamma2) so the 28 input slices reach the 16 HWDGE
   queues ~2x sooner (Sync-issued DMAs cost ~565ns of issue time each).
 - half-tile A-stage (range reduction r01/r23) + 2-channel packed ACT sins
   for earlier pipeline starts; split output DMA.

Sharding: pure data parallelism over the event axis N=262144 across the 8
NeuronCores (32768 events each, laid out [128 partitions x 256 events] with
the R=4 resonance slices side by side in the free axis).
"""

import numpy as np

import concourse.bass as bass
from concourse import bacc, mybir
from concourse.bass_utils import run_bass_kernel_spmd

F32 = mybir.dt.float32
BF16 = mybir.dt.bfloat16
FP16 = mybir.dt.float16
I16 = mybir.dt.int16
ALU = mybir.AluOpType
ACTF = mybir.ActivationFunctionType

R = 4
N_TOTAL = 262144
N_CORES = 8
N_CORE = N_TOTAL // N_CORES     # 32768 events per core
P = 128                         # SBUF partitions
E = N_CORE // P                 # 256 events per partition per resonance
W = R * E                       # 1024 free-dim of a full working tile
H = W // 2                      # half tile (r01 / r23)

MAGIC = float(np.float32(1.5 * 2.0**23))   # round-to-nearest-int bias trick
INV4PI = float(np.float32(1.0 / (4.0 * np.pi)))
TWOPI = float(np.float32(2.0 * np.pi))
HALFPI = float(np.float32(np.pi / 2.0))

INPUT_NAMES = ("alpha1", "beta1", "gamma1", "alpha2", "beta2", "gamma2", "m")


def _register_custom_ops():
    import concourse.dve_ops as dve_ops
    from concourse.dve_spec import Spec, Src0, Src1, C0, C1, C2, sq, lower, _has_src1
    from concourse.dve_uop import DveOpSpec
    from concourse.dve_ops import DveOp

    if any(op.name == "ANT_RANGE_RED_ADD" for op in dve_ops.OPS):
        return {op.name: op for op in dve_ops.OPS}

    def make_op(name, spec):
        shas = {}
        for ver in ("v3", "v4"):
            uops = lower(spec, ver=ver)
            shas[ver] = DveOpSpec(name=name, opcode=31, uops=uops,
                                  rd1_en=_has_src1(spec)).sha(ver)
        return DveOp(name, spec, subdim=False, uops_sha=shas)

    def _rr_ref(sgn):
        def ref(in0, in1, s0, s1, imm2):
            t = ((in0 + sgn * in1) * s0 + s1).astype(np.float32)
            r = ((t + imm2).astype(np.float32) - imm2).astype(np.float32)
            return (t - r).astype(np.float32)
        return ref

    u = (Src0 + Src1) * C0 + C1
    rr_add = make_op("ANT_RANGE_RED_ADD",
                     Spec(body=u - ((u + C2) - C2), reference=_rr_ref(1.0)))
    u2 = (Src0 - Src1) * C0 + C1
    rr_sub = make_op("ANT_RANGE_RED_SUB",
                     Spec(body=u2 - ((u2 + C2) - C2), reference=_rr_ref(-1.0)))
    chi = make_op("ANT_CHI", Spec(
        body=sq(Src0) * C0 + sq(Src1) * C1 + C2,
        reference=lambda in0, in1, s0, s1, imm2:
            (in0 * in0 * s0 + in1 * in1 * s1 + imm2).astype(np.float32)))
    den = make_op("ANT_DEN", Spec(
        body=sq(C0 - Src0) + C1,
        reference=lambda in0, in1, s0, s1, imm2:
            ((s0 - in0) * (s0 - in0) + s1).astype(np.float32)))

    for op in (rr_add, rr_sub, chi, den):
        dve_ops.OPS.append(op)
        dve_ops._SUB_OPCODE_FOR_NAME[op.name] = (
            dve_ops._CUSTOM_DVE_ROW_BASE + len(dve_ops.OPS) - 1)
        dve_ops.CUSTOM_DVE_SPECS[op.name] = op.spec
    assert max(dve_ops._SUB_OPCODE_FOR_NAME.values()) < 0x20
    return {op.name: op for op in dve_ops.OPS}


def _rs(r):
    return slice(r * E, (r + 1) * E)


def build(m0, g0, coef_r, coef_i):
    OPS = _register_custom_ops()
    RR_ADD, RR_SUB, CHI, DEN = (OPS["ANT_RANGE_RED_ADD"], OPS["ANT_RANGE_RED_SUB"],
                                OPS["ANT_CHI"], OPS["ANT_DEN"])
    AT = FP16

    nc = bacc.Bacc("TRN2", target_bir_lowering=False, debug=False,
                   num_devices=N_CORES)
    ins = {k: nc.dram_tensor(k, (R, N_CORE),
                             F32 if k == "m" else FP16,
                             kind="ExternalInput").ap()
           for k in INPUT_NAMES}
    out_ap = nc.dram_tensor("out", (N_CORE,), F32, kind="ExternalOutput").ap()

    f32 = np.float32
    m0 = m0.astype(np.float64); g0 = g0.astype(np.float64)
    cR = [float(f32(coef_r[r] * np.cos(coef_i[r]))) for r in range(R)]
    cI = [float(f32(coef_r[r] * np.sin(coef_i[r]))) for r in range(R)]
    m0sq = [float(f32(m0[r] * m0[r])) for r in range(R)]
    y = [float(f32(m0[r] * g0[r])) for r in range(R)]
    ysq = [float(f32(f32(y[r]) * f32(y[r]))) for r in range(R)]
    k1 = [float(f32(-f32(cI[r]) * f32(y[r]))) for r in range(R)]
    k2 = [float(f32(f32(cR[r]) * f32(y[r]))) for r in range(R)]
    c27 = [float(f32(7.0 * (f32(cR[r])**2 + f32(cI[r])**2))) for r in range(R)]

    # ---- static SBUF allocation ----
    alloc = []
    def sb(name, shape, dt=F32):
        t = nc.alloc_sbuf_tensor(name, list(shape), dt)
        alloc.append(t)
        return t.ap()

    tin = {k: sb(f"in_{k}", [P, W], F32 if k == "m" else FP16)
           for k in INPUT_NAMES}
    pi2 = sb("pi2", [P, 1])
    ub = sb("ub", [P, W], AT); vb = sb("vb", [P, W], AT)
    wb = sb("wb", [P, W], AT); zb = sb("zb", [P, W], AT)
    # packed fracs [fA|fB|fC|fD] and their absolute values, fp16
    f4 = sb("f4", [P, 4 * W], AT)
    a4 = sb("a4", [P, 4 * W], AT)
    fA = f4[:, 0:W]; fB = f4[:, W:2*W]; fC = f4[:, 2*W:3*W]; fD = f4[:, 3*W:4*W]
    aA = a4[:, 0:W]; aB = a4[:, W:2*W]; aC = a4[:, 2*W:3*W]; aD = a4[:, 3*W:4*W]
    # sct tiles (sin/cos of composite angles), packed [As|Bs|Cs|Ds] / [Ac|..]
    sct_s4 = sb("sct_s4", [P, 4 * W], AT)
    sct_c4 = sb("sct_c4", [P, 4 * W], AT)
    sct = {"As": sct_s4[:, 0:W], "Bs": sct_s4[:, W:2*W],
           "Cs": sct_s4[:, 2*W:3*W], "Ds": sct_s4[:, 3*W:4*W],
           "Ac": sct_c4[:, 0:W], "Bc": sct_c4[:, W:2*W],
           "Cc": sct_c4[:, 2*W:3*W], "Dc": sct_c4[:, 3*W:4*W]}
    B1P = sb("B1P", [P, 2 * W], AT)   # [cb1|sb1]
    B2P = sb("B2P", [P, 2 * W], AT)   # [cb2|sb2]
    cb1 = B1P[:, 0:W]; sb1 = B1P[:, W:2*W]
    cb2 = B2P[:, 0:W]; sb2_ = B2P[:, W:2*W]
    MS2ab = sb("MS2ab", [P, 2 * W], AT)   # [M0|M1]
    MS2cd = sb("MS2cd", [P, 2 * W], AT)   # [M2|M3]
    PQ4ab = sb("PQ4ab", [P, 4 * W], AT)   # [pqAs|pqBs|pqAc|pqBc]
    PQ4cd = sb("PQ4cd", [P, 4 * W], AT)   # [pqCs|pqDs|pqCc|pqDc]
    PQsab = PQ4ab[:, 0:2*W]; PQcab = PQ4ab[:, 2*W:4*W]
    PQscd = PQ4cd[:, 0:2*W]; PQccd = PQ4cd[:, 2*W:4*W]
    AB4 = sb("AB4", [P, 4 * W], AT)   # [are|aim|bre|bim]
    are = AB4[:, 0:W]; aim = AB4[:, W:2*W]
    bre = AB4[:, 2*W:3*W]; bim = AB4[:, 3*W:4*W]
    msq = sb("msq", [P, W]); den = sb("den", [P, W]); rcp = den
    wp1 = sb("wp1", [P, W], AT); wp2 = sb("wp2", [P, W], AT)
    rcph = sb("rcph", [P, W], AT)
    wre = sb("wre", [P, W]); wim = sb("wim", [P, W])
    WH = sb("WH", [P, 2 * W], BF16)   # [wreh|wimh]
    wreh = WH[:, 0:W]; wimh = WH[:, W:2*W]
    dall = sb("dall", [P, W], AT); dh = sb("dh", [P, 2 * E], AT)
    dg = sb("dg", [P, E], AT)
    acc = sb("acc", [P, E])
    # pair scratch (reused across the 3 shift groups; DVE program order)
    NP3 = 3 * E
    PT = sb("PT", [P, 4 * NP3], AT)    # 4 packed products
    SD = sb("SD", [P, 4 * NP3], AT)    # [S1ch0|S1ch1|DTch0|DTch1]
    S1 = SD[:, 0:2*NP3]; DT = SD[:, 2*NP3:4*NP3]
    QT = sb("QT", [P, 4 * NP3], AT)    # [q1|q4|q2|q3]
    NP6 = 6 * E
    AVI2 = sb("AVI2", [P, 2 * NP6], AT)   # [avr|avi]
    avr = AVI2[:, 0:NP6]; avi = AVI2[:, NP6:2*NP6]
    chis = sb("chis", [P, NP6], BF16); gw = sb("gw", [P, NP6], BF16)
    GT6 = sb("GT6", [P, 2 * NP6], BF16)
    term = sb("term", [P, NP6], BF16)

    sem_aa = [nc.alloc_semaphore(f"s_aa{h}") for h in range(2)]  # alpha pair per half
    sem_gg = [nc.alloc_semaphore(f"s_gg{h}") for h in range(2)]  # gamma pair per half
    with (
        nc.semaphore("s_b1") as sem_b1,
        nc.semaphore("s_b2") as sem_b2,
        nc.semaphore("s_m") as sem_m,
        nc.semaphore("s_out") as sem_out,
        nc.semaphore("act_sem") as act_sem,
        nc.semaphore("vec_sem") as vec_sem,
        nc.Block() as block,
    ):
        def dma(eng, k, h, sem):
            """half-tensor DMA: resonances 2h..2h+1 into tin column-half h."""
            eng.dma_start(
                tin[k][:, h * H:(h + 1) * H].rearrange("p (r e) -> p r e",
                                                       r=2, e=E),
                ins[k][2*h:2*h+2].rearrange("r (p e) -> p r e", p=P, e=E),
            ).then_inc(sem, 16)

        # ------------- SYNC ------------------------------------------------
        @block.sync
        def _(sync):
            dma(sync, "alpha1", 0, sem_aa[0])
            dma(sync, "alpha1", 1, sem_aa[1])
            dma(sync, "gamma1", 1, sem_gg[1])
            dma(sync, "m", 0, sem_m)
            dma(sync, "m", 1, sem_m)
            # split output: two column-halves of the (p, e) view
            outv = out_ap.rearrange("(p e) -> p e", p=P, e=E)
            sync.wait_ge(vec_sem, 17)
            sync.dma_start(outv[:, 0:E//2], acc[:, 0:E//2]).then_inc(sem_out, 16)
            sync.wait_ge(sem_out, 32)

        # ------------- GPSIMD ----------------------------------------------
        @block.gpsimd
        def _(gpsimd):
            dma(gpsimd, "gamma1", 0, sem_gg[0])
            dma(gpsimd, "gamma2", 0, sem_gg[0])
            dma(gpsimd, "alpha2", 1, sem_aa[1])
            dma(gpsimd, "beta2", 0, sem_b2)
            dma(gpsimd, "beta2", 1, sem_b2)

        # ------------- SCALAR (ACT): beta1 + gamma r2 DMAs, transcendentals --
        # act_sem: 1 cb1, 2 sb1, 3 cb2, 4 sb2,
        #          per half h: 5+3h AsBs, 6+3h CsDs, 7+3h cos4
        #          11 msq, 12 wreh, 13 wimh
        @block.scalar
        def _(scalar):
            dma(scalar, "alpha2", 0, sem_aa[0])
            dma(scalar, "beta1", 0, sem_b1)
            dma(scalar, "beta1", 1, sem_b1)
            dma(scalar, "gamma2", 1, sem_gg[1])
            scalar.wait_ge(vec_sem, 1)   # pi2 memset
            scalar.wait_ge(sem_b1, 32)
            scalar.activation(cb1[:], tin["beta1"][:], ACTF.Sin, scale=0.5,
                              bias=pi2[:]).then_inc(act_sem, 1)        # 1
            scalar.activation(sb1[:], tin["beta1"][:], ACTF.Sin,
                              scale=0.5).then_inc(act_sem, 1)          # 2
            scalar.wait_ge(sem_b2, 32)
            scalar.activation(cb2[:], tin["beta2"][:], ACTF.Sin, scale=0.5,
                              bias=pi2[:]).then_inc(act_sem, 1)        # 3
            scalar.activation(sb2_[:], tin["beta2"][:], ACTF.Sin,
                              scale=0.5).then_inc(act_sem, 1)          # 4

            s4v = sct_s4.rearrange("p (c w) -> p c w", c=4, w=W)
            c4v = sct_c4.rearrange("p (c w) -> p c w", c=4, w=W)
            f4v = f4.rearrange("p (c w) -> p c w", c=4, w=W)
            a4v = a4.rearrange("p (c w) -> p c w", c=4, w=W)
            for h in range(2):
                s = slice(h * H, h * H + H)
                scalar.wait_ge(vec_sem, 3 + 6 * h)    # fA,fB half h
                scalar.activation(s4v[:, 0:2, s], f4v[:, 0:2, s], ACTF.Sin,
                                  scale=TWOPI).then_inc(act_sem, 1)   # 5/9
                scalar.wait_ge(vec_sem, 7 + 6 * h)    # abs half h (all four)
                scalar.activation(c4v[:, :, s], a4v[:, :, s], ACTF.Sin,
                                  scale=-TWOPI, bias=pi2[:]).then_inc(act_sem, 1)  # 6/9 cos4
                scalar.activation(s4v[:, 2:4, s], f4v[:, 2:4, s], ACTF.Sin,
                                  scale=-TWOPI).then_inc(act_sem, 1)  # 7/10 CsDs

            scalar.wait_ge(sem_m, 32)
            scalar.activation(msq[:], tin["m"][:], ACTF.Square).then_inc(act_sem, 1)  # 11
            for r in range(R):
                scalar.activation(wp1[:, _rs(r)], msq[:, _rs(r)], ACTF.Copy,
                                  scale=-cR[r],
                                  bias=float(f32(cR[r]*m0sq[r] + k1[r]))
                                  ).then_inc(act_sem, 1)               # 12..15
            for r in range(R):
                scalar.activation(wp2[:, _rs(r)], msq[:, _rs(r)], ACTF.Copy,
                                  scale=-cI[r],
                                  bias=float(f32(cI[r]*m0sq[r] + k2[r]))
                                  ).then_inc(act_sem, 1)               # 16..19
            scalar.wait_ge(vec_sem, 14)   # rcp
            for r in range(R):
                scalar.activation(dall[:, _rs(r)], rcp[:, _rs(r)], ACTF.Copy,
                                  scale=c27[r]).then_inc(act_sem, 1)   # 20..23
            outv2 = out_ap.rearrange("(p e) -> p e", p=P, e=E)
            scalar.wait_ge(vec_sem, 18)
            scalar.dma_start(outv2[:, E//2:], acc[:, E//2:]).then_inc(sem_out, 16)

        # ------------- VECTOR (DVE) -------------
        # vec_sem: 1 memset; per half h: 3+6h fAB, 5+6h fCD, 7+6h abs
        #   (h=0: 2..3 used as (2=fA.. we inc after fB), see below)
        # 14 wre, 15 wim, 16/17 acc halves
        @block.vector
        def _(vector):
            nc.vector.memset(pi2[:], HALFPI).then_inc(vec_sem, 1)  # 1

            # ---- stage A: half-tile pre-adds + half-tile RR + packed abs ----
            for h in range(2):
                s = slice(h * H, h * H + H)
                vector.wait_ge(sem_aa[h], 32)
                nc.vector.tensor_add(ub[:, s], tin["alpha1"][:, s], tin["alpha2"][:, s])
                nc.vector.tensor_sub(vb[:, s], tin["alpha1"][:, s], tin["alpha2"][:, s])
                vector.wait_ge(sem_gg[h], 32)
                nc.vector.tensor_add(wb[:, s], tin["gamma1"][:, s], tin["gamma2"][:, s])
                nc.vector.tensor_sub(zb[:, s], tin["gamma1"][:, s], tin["gamma2"][:, s])
                nc.vector._custom_dve(RR_ADD, out=fA[:, s], in0=ub[:, s],
                                      in1=wb[:, s], s0=INV4PI, s1=0.0, imm2=MAGIC)
                nc.vector._custom_dve(RR_SUB, out=fB[:, s], in0=vb[:, s],
                                      in1=zb[:, s], s0=INV4PI, s1=0.0,
                                      imm2=MAGIC).then_inc(vec_sem, 2)   # 3/9
                nc.vector._custom_dve(RR_ADD, out=fC[:, s], in0=ub[:, s],
                                      in1=zb[:, s], s0=INV4PI, s1=0.0, imm2=MAGIC)
                nc.vector._custom_dve(RR_SUB, out=fD[:, s], in0=vb[:, s],
                                      in1=wb[:, s], s0=INV4PI, s1=0.0,
                                      imm2=MAGIC).then_inc(vec_sem, 2)   # 5/11
                # packed |f| for all four combos of this half: 4-ch AP view
                f4i = f4.bitcast(I16).rearrange("p (c w) -> p c w", c=4, w=W)
                a4i = a4.bitcast(I16).rearrange("p (c w) -> p c w", c=4, w=W)
                nc.vector.tensor_scalar(a4i[:, :, s], f4i[:, :, s], 0x7FFF, None,
                                        ALU.bitwise_and).then_inc(vec_sem, 2)  # 7/13

            # ---- stage B: Wigner magnitudes + packed 2-ch pq products ----
            vector.wait_ge(act_sem, 4)
            ms_ab = MS2ab.rearrange("p (c w) -> p c w", c=2, w=W)
            ms_cd = MS2cd.rearrange("p (c w) -> p c w", c=2, w=W)
            b1v = B1P.rearrange("p (c w) -> p c w", c=2, w=W)
            b2v = B2P.rearrange("p (c w) -> p c w", c=2, w=W)
            nc.vector.tensor_mul(ms_ab[:], b1v[:], b2v[:])            # [M0|M1]
            nc.vector.tensor_mul(ms_cd[:], b1v[:], b2v[:, 1::-1, :])  # [M2|M3]
            s4vv = sct_s4.rearrange("p (c w) -> p c w", c=4, w=W)
            c4vv = sct_c4.rearrange("p (c w) -> p c w", c=4, w=W)
            psab = PQsab.rearrange("p (c w) -> p c w", c=2, w=W)
            pcab = PQcab.rearrange("p (c w) -> p c w", c=2, w=W)
            pscd = PQscd.rearrange("p (c w) -> p c w", c=2, w=W)
            pccd = PQccd.rearrange("p (c w) -> p c w", c=2, w=W)
            pq4ab_v = PQ4ab.rearrange("p (c w) -> p c w", c=4, w=W)
            pq4cd_v = PQ4cd.rearrange("p (c w) -> p c w", c=4, w=W)
            ab4v = AB4.rearrange("p (c w) -> p c w", c=4, w=W)
            for h in range(2):
                s = slice(h * H, h * H + H)
                vector.wait_ge(act_sem, 5 + 3 * h)   # AsBs half h
                nc.vector.tensor_mul(psab[:, :, s], ms_ab[:, :, s], s4vv[:, 0:2, s])
                vector.wait_ge(act_sem, 6 + 3 * h)   # cos4 half h
                nc.vector.tensor_mul(pcab[:, :, s], ms_ab[:, :, s], c4vv[:, 0:2, s])
                # [are|aim] = [Ac-Bc | Bs-As]: L=ch[2,1], R=ch[3,0]
                nc.vector.tensor_sub(ab4v[:, 0:2, s], pq4ab_v[:, 2:0:-1, s],
                                     pq4ab_v[:, 3::-3, s])
                nc.vector.tensor_mul(pccd[:, :, s], ms_cd[:, :, s], c4vv[:, 2:4, s])
                vector.wait_ge(act_sem, 7 + 3 * h)   # CsDs half h
                nc.vector.tensor_mul(pscd[:, :, s], ms_cd[:, :, s], s4vv[:, 2:4, s])
                # [bre|bim] = [Cc+Dc | Cs+Ds]: L=ch[2,0], R=ch[3,1]
                nc.vector.tensor_add(ab4v[:, 2:4, s], pq4cd_v[:, 2::-2, s],
                                     pq4cd_v[:, 3::-2, s])

            # ---- stage C: Breit-Wigner ----
            vector.wait_ge(act_sem, 11)   # msq
            for r in range(R):
                nc.vector._custom_dve(DEN, out=den[:, _rs(r)], in0=msq[:, _rs(r)],
                                      s0=m0sq[r], s1=ysq[r])
            nc.vector.reciprocal_approx_fast(out=rcp[:], in_=den[:]) \
                .then_inc(vec_sem, 1)   # 14
            nc.vector.tensor_copy(rcph[:], rcp[:])
            vector.wait_ge(act_sem, 15)
            nc.vector.tensor_mul(wreh[:], wp1[:], rcph[:]).then_inc(vec_sem, 1)  # 15
            vector.wait_ge(act_sem, 19)
            nc.vector.tensor_mul(wimh[:], wp2[:], rcph[:]).then_inc(vec_sem, 1)  # 16
            vector.wait_ge(act_sem, 23)   # dall (7|coef|^2 * rcp on ACT)
            nc.vector.tensor_add(dh[:], dall[:, 0:2*E], dall[:, 2*E:4*E])
            nc.vector.tensor_add(dg[:], dh[:, 0:E], dh[:, E:2*E])

            # ---- stage D: pairs ----
            ab4v = AB4.rearrange("p (c w) -> p c w", c=4, w=W)
            whv = WH.rearrange("p (c w) -> p c w", c=2, w=W)
            gt6v = GT6.rearrange("p (c w) -> p c w", c=2, w=NP6)
            goffs = {1: 0, 2: 3 * E, 3: 5 * E}
            for sig in (1, 2, 3):
                n = (R - sig) * E
                go = goffs[sig]
                L = slice(0, n)
                Rr = slice(sig * E, sig * E + n)
                ptv = PT.rearrange("p (c w) -> p c w", c=4, w=NP3)
                sd4v = SD.rearrange("p (c w) -> p c w", c=4, w=NP3)
                av2v = AVI2.rearrange("p (c w) -> p c w", c=2, w=NP6)
                nc.vector.tensor_mul(ptv[:, :, :n], ab4v[:, :, L], ab4v[:, :, Rr])
                nc.vector.tensor_add(sd4v[:, 0:2, :n],
                                     ptv[:, 0:2, :n], ptv[:, 2:4, :n])
                qtv = QT.rearrange("p (c w) -> p c w", c=4, w=NP3)
                ab2a = ab4v[:, 0:2, :]      # [are|aim]
                ab2af = ab4v[:, 1::-1, :]   # [aim|are]
                ab2b = ab4v[:, 2:4, :]      # [bre|bim]
                ab2bf = ab4v[:, 3:1:-1, :]  # [bim|bre]
                nc.vector.tensor_mul(qtv[:, 0::2, :n], ab2a[:, :, L],
                                     ab2af[:, :, Rr])   # q1, q2
                nc.vector.tensor_mul(qtv[:, 1::2, :n], ab2bf[:, :, L],
                                     ab2b[:, :, Rr])    # q4, q3
                nc.vector.tensor_sub(sd4v[:, 2:4, :n],
                                     qtv[:, 0:2, :n], qtv[:, 2:4, :n])
                # [avr|avi] in one 2-ch add: ch0 = S1c0+S1c1, ch1 = DTc0+DTc1
                nc.vector.tensor_add(av2v[:, :, go:go+n],
                                     sd4v[:, 0::2, :n], sd4v[:, 1::2, :n])
                nc.vector.tensor_mul(gt6v[:, :, go:go+n], whv[:, :, L], whv[:, :, Rr])
            nc.vector._custom_dve(CHI, out=chis[:], in0=avr[:], in1=avi[:],
                                  s0=20.0, s1=4.0, imm2=-6.0)
            nc.vector.tensor_add(gw[:], GT6[:, 0:NP6], GT6[:, NP6:2*NP6])
            nc.vector.tensor_mul(term[:], chis[:], gw[:])
            # tree-sum the 6 pair blocks: one packed 3-pair add, then merge
            tv = term.rearrange("p (c w) -> p c w", c=6, w=E)
            s3 = DT.rearrange("p (c w) -> p c w", c=2, w=NP3)  # reuse DT scratch
            nc.vector.tensor_tensor(s3[:, 0, 0:3*E].rearrange("p (c w) -> p c w", c=3, w=E),
                                    tv[:, 0::2, :], tv[:, 1::2, :], ALU.add)
            nc.vector.tensor_add(DT[:, 3*E:4*E], DT[:, 0:E], DT[:, E:2*E])
            nc.vector.tensor_add(dh[:, 0:E], DT[:, 2*E:3*E], DT[:, 3*E:4*E])
            # final: acc = pairs + diagonal, split for output DMA overlap
            nc.vector.tensor_add(acc[:, 0:E//2], dh[:, 0:E//2], dg[:, 0:E//2]) \
                .then_inc(vec_sem, 1)   # 17
            nc.vector.tensor_add(acc[:, E//2:E], dh[:, E//2:E], dg[:, E//2:E]) \
                .then_inc(vec_sem, 1)   # 18

    nc.compile()
    return nc


_CACHE = {}


def kernel(alpha1, beta1, gamma1, alpha2, beta2, gamma2, m, m0, g0,
           coef_r, coef_i, _want_trace=False):
    key = (np.asarray(m0, np.float32).tobytes(), np.asarray(g0, np.float32).tobytes(),
           np.asarray(coef_r, np.float32).tobytes(), np.asarray(coef_i, np.float32).tobytes())
    if key not in _CACHE:
        _CACHE[key] = build(np.asarray(m0, np.float32), np.asarray(g0, np.float32),
                            np.asarray(coef_r, np.float32), np.asarray(coef_i, np.float32))
    nc = _CACHE[key]
    full = {"alpha1": alpha1, "beta1": beta1, "gamma1": gamma1,
            "alpha2": alpha2, "beta2": beta2, "gamma2": gamma2, "m": m}
    in_maps = []
    for i in range(N_CORES):
        sl = slice(i * N_CORE, (i + 1) * N_CORE)
        in_maps.append({k: np.ascontiguousarray(
                            np.asarray(v, np.float32)[:, sl].astype(
                                np.float32 if k == "m" else np.float16))
                        for k, v in full.items()})
    res = run_bass_kernel_spmd(nc, in_maps, core_ids=list(range(N_CORES)),
                               trace=_want_trace)
    out = np.concatenate([res.results[i]["out"] for i in range(N_CORES)])
    if _want_trace:
        kernel._last_result = res
    return out.astype(np.float32)
